# revision 1
# baseline (speedup 1.0000x reference)
"""nn_Downsample: depthwise 4x4 stride-2 pad-1 blur downsample on 8 NeuronCores.

Input  x [16, 256, 256, 256] fp32 (NCHW), kernel [4, 4] fp32 (rank-1 FIR).
Output   [16, 256, 128, 128] fp32.

Sharding: pure data parallelism - 2 samples per core across 8 cores.

Per-core program: the conv is separable (kernel = fh x fw outer product),
computed per 256x256 plane as two TensorEngine matmul stages:
  stage 1 (contract h): tmpT[w, i] = sum_h x[h, w] * AT[h, i]
  stage 2 (contract w): out[i, j]  = sum_w tmpT[w, i] * B[w, j]
with AT/B banded matrices holding the taps. fp32r matmuls with the
[W0|W1]/[W1|W0] rhs concat trick keep the moving dim at 256 (full PE rate);
the two accumulation groups for adjacent w-blocks write overlapping PSUM
ranges so their valid halves land adjacent and one [128,256] DVE copy
replaces two [128,128] copies (same for paired channels in stage 2).

DMA routing: all input loads ride the sync HWDGE ring, all output stores the
scalar HWDGE ring, PSUM->SBUF copies run on DVE only. Mixing loads+stores on
one ring (or putting copies on ACT, which also issues stores) head-of-line
blocks the input stream; measured ~8% slower.
"""

import sys

sys.path.insert(0, "/opt/trn_rl_repo")

import numpy as np

N_CORES = 8

# Final configuration (picked by A/B timing on hardware; see work/exp.py)
CFG = dict(
    c_group=8,
    copy_policy="vec",
    in_dma="sync",
    out_dma="scalar",
    batch_copies=True,
    split_in=2,
    in_qw_merge=True,
    xbufs=6,
    obufs=6,
    tbufs=8,
    ps1bufs=4,
    ps2bufs=4,
)

_RUNNER_CACHE = {}


def _factor_kernel(k):
    k = np.asarray(k, dtype=np.float64)
    canon = np.outer([1.0, 3.0, 3.0, 1.0], [1.0, 3.0, 3.0, 1.0]) / 64.0
    if np.allclose(k, canon, rtol=1e-5, atol=1e-8):
        f = np.array([1.0, 3.0, 3.0, 1.0]) / 8.0
        return f, f
    u, s, vt = np.linalg.svd(k)
    fh = u[:, 0] * np.sqrt(s[0])
    fw = vt[0] * np.sqrt(s[0])
    if fh.sum() < 0:
        fh, fw = -fh, -fw
    return fh, fw


def _band_matrices(fh, fw, H=256, W=256):
    HO, WO = H // 2, W // 2
    AT = np.zeros((H, HO), dtype=np.float32)
    for i in range(HO):
        for a in range(4):
            h = 2 * i - 1 + a
            if 0 <= h < H:
                AT[h, i] = fh[a]
    B = np.zeros((W, WO), dtype=np.float32)
    for j in range(WO):
        for b in range(4):
            w = 2 * j - 1 + b
            if 0 <= w < W:
                B[w, j] = fw[b]
    return AT, B


def _weight_inputs(kernel):
    fh, fw = _factor_kernel(kernel)
    AT, B = _band_matrices(fh, fw)
    B0, B1 = B[:128], B[128:]
    ATe, ATo = AT[0::2], AT[1::2]
    return {
        "AT01": np.ascontiguousarray(np.concatenate([ATe, ATo], axis=1)),
        "AT10": np.ascontiguousarray(np.concatenate([ATo, ATe], axis=1)),
        "B01": np.ascontiguousarray(np.concatenate([B0, B1], axis=1)),
        "B10": np.ascontiguousarray(np.concatenate([B1, B0], axis=1)),
    }


def _build_nc(
    *,
    loop_iters=None,
    c_group=8,
    copy_policy="vec",
    in_dma="sync",
    out_dma="scalar",
    batch_copies=True,
    split_in=1,
    in_qw_merge=True,
    xbufs=4,
    obufs=4,
    tbufs=8,
    ps1bufs=4,
    ps2bufs=4,
    n_samples=2,
    C=256,
):
    import concourse.mybir as mybir
    import concourse.tile as tile
    from concourse import bacc

    F32 = mybir.dt.float32
    F32R = mybir.dt.float32r

    H = W = 256
    HO = WO = 128
    s1_dt = F32R
    s2_dt = F32R
    timing = loop_iters is not None

    nc = bacc.Bacc("TRN2", target_bir_lowering=False)
    if timing:
        x_d = nc.dram_tensor("x", [n_samples, C, H, W], F32, kind="Internal")
        y_d = nc.dram_tensor("y", [n_samples, C, HO, WO], F32, kind="Internal")
        done_d = nc.dram_tensor("done", [1, 1], F32, kind="ExternalOutput")
    else:
        x_d = nc.dram_tensor("x", [n_samples, C, H, W], F32, kind="ExternalInput")
        y_d = nc.dram_tensor("y", [n_samples, C, HO, WO], F32,
                             kind="ExternalOutput")

    wnames = ["AT01", "AT10", "B01", "B10"]
    w_d = {n: nc.dram_tensor(n, [128, 256], F32, kind="ExternalInput")
           for n in wnames}

    with tile.TileContext(nc) as tc:
        with (
            tc.tile_pool(name="wpool", bufs=1) as wpool,
            tc.tile_pool(name="xpool", bufs=xbufs) as xpool,
            tc.tile_pool(name="tpool", bufs=tbufs) as tpool,
            tc.tile_pool(name="opool", bufs=obufs) as opool,
            tc.tile_pool(name="ps1", bufs=ps1bufs, space="PSUM") as ps1pool,
            tc.tile_pool(name="ps2", bufs=ps2bufs, space="PSUM") as ps2pool,
        ):
            wt = {}
            for n in wnames:
                wdt = s1_dt if n.startswith("AT") else s2_dt
                t = wpool.tile([128, 256], wdt, tag=f"w_{n}", name=f"w_{n}")
                nc.gpsimd.dma_start(out=t[:], in_=w_d[n][:].bitcast(wdt))
                wt[n] = t

            eng_i = 0

            def copy_tile(out_ap, in_ap, kind="tmp"):
                nonlocal eng_i
                if copy_policy == "vec":
                    nc.vector.tensor_copy(out=out_ap, in_=in_ap)
                elif copy_policy == "vec_out_scalar":
                    if kind == "out":
                        nc.scalar.copy(out_ap, in_ap)
                    else:
                        nc.vector.tensor_copy(out=out_ap, in_=in_ap)
                elif copy_policy == "alt":
                    if eng_i % 2 == 0:
                        nc.vector.tensor_copy(out=out_ap, in_=in_ap)
                    else:
                        nc.scalar.copy(out_ap, in_ap)
                    eng_i += 1
                else:
                    raise ValueError(copy_policy)

            def in_engine(cg):
                if in_dma == "alt":
                    return nc.sync if cg % 2 == 0 else nc.scalar
                return getattr(nc, in_dma)

            def out_engine(cg):
                if out_dma == "alt":
                    return nc.sync if cg % 2 == 0 else nc.scalar
                return getattr(nc, out_dma)

            def body():
                for n in range(n_samples):
                    for cg in range(C // c_group):
                        c0 = cg * c_group
                        xt = xpool.tile([128, c_group * 2 * W], s1_dt, tag="X",
                                        name=f"x_{n}_{cg}")
                        cpp = c_group // split_in
                        for sp in range(split_in):
                            cl, ch = sp * cpp, (sp + 1) * cpp
                            src = x_d[n, c0 + cl : c0 + ch, :, :]
                            if in_qw_merge:
                                # explicit (q w) merge: the AP normalizer
                                # does not coalesce the two adjacent rows
                                # itself; 2KB descriptors measure ~5% faster
                                src = src.rearrange(
                                    "c (p q) w -> p c (q w)", q=2)
                            else:
                                src = src.rearrange(
                                    "c (p q) w -> p c q w", q=2)
                            in_engine(cg).dma_start(
                                out=xt[:, cl * 2 * W : ch * 2 * W],
                                in_=src.bitcast(s1_dt))

                        out_t = opool.tile([128, c_group * WO], F32, tag="OUT",
                                           name=f"o_{n}_{cg}")

                        if batch_copies:
                            for cp in range(c_group // 2):
                                po = ps2pool.tile([128, 384], F32, tag="ps2",
                                                  name=f"p2_{n}_{cg}_{cp}")
                                for ci in range(2):
                                    c = 2 * cp + ci
                                    pst = ps1pool.tile([128, 384], F32,
                                                       tag="ps1",
                                                       name=f"p1_{n}_{cg}_{c}")
                                    for blk in range(2):
                                        base = c * 2 * W + blk * 128
                                        lhsE = xt[:, base : base + 128]
                                        lhsO = xt[:, base + W : base + W + 128]
                                        dst = pst[:, blk * 128 : blk * 128 + 256]
                                        nc.tensor.matmul(dst, lhsE,
                                                         wt["AT01"][:],
                                                         start=True, stop=False)
                                        nc.tensor.matmul(dst, lhsO,
                                                         wt["AT10"][:],
                                                         start=False, stop=True)
                                    tmpT = tpool.tile([128, 256], s2_dt,
                                                      tag="tmpT",
                                                      name=f"t_{n}_{cg}_{c}")
                                    copy_tile(tmpT[:], pst[:, 0:256])

                                    dst2 = po[:, ci * 128 : ci * 128 + 256]
                                    nc.tensor.matmul(dst2, tmpT[:, 0:128],
                                                     wt["B01"][:],
                                                     start=True, stop=False)
                                    nc.tensor.matmul(dst2, tmpT[:, 128:256],
                                                     wt["B10"][:],
                                                     start=False, stop=True)
                                c0p = 2 * cp * WO
                                copy_tile(out_t[:, c0p : c0p + 2 * WO],
                                          po[:, 0:256], kind="out")
                        else:
                            for c in range(c_group):
                                pst = ps1pool.tile([128, 512], F32, tag="ps1",
                                                   name=f"p1_{n}_{cg}_{c}")
                                for blk in range(2):
                                    base = c * 2 * W + blk * 128
                                    lhsE = xt[:, base : base + 128]
                                    lhsO = xt[:, base + W : base + W + 128]
                                    dst = pst[:, blk * 256 : blk * 256 + 256]
                                    nc.tensor.matmul(dst, lhsE, wt["AT01"][:],
                                                     start=True, stop=False)
                                    nc.tensor.matmul(dst, lhsO, wt["AT10"][:],
                                                     start=False, stop=True)
                                tmpT = tpool.tile([128, 256], s2_dt, tag="tmpT",
                                                  name=f"t_{n}_{cg}_{c}")
                                copy_tile(tmpT[:, 0:128], pst[:, 0:128])
                                copy_tile(tmpT[:, 128:256], pst[:, 256:384])

                                po = ps2pool.tile([128, 256], F32, tag="ps2",
                                                  name=f"p2_{n}_{cg}_{c}")
                                nc.tensor.matmul(po[:], tmpT[:, 0:128],
                                                 wt["B01"][:],
                                                 start=True, stop=False)
                                nc.tensor.matmul(po[:], tmpT[:, 128:256],
                                                 wt["B10"][:],
                                                 start=False, stop=True)

                                copy_tile(out_t[:, c * WO : c * WO + WO],
                                          po[:, 0:128], kind="out")

                        dsty = y_d[n, c0 : c0 + c_group, :, :].rearrange(
                            "c i j -> i c j")
                        out_engine(cg).dma_start(out=dsty, in_=out_t[:])

            if timing:
                if loop_iters > 1:
                    # The body spans many IRAM blocks per engine; without
                    # branch hints every back-edge pays a ~3-4us I$-miss
                    # stall that the single-shot (graded) kernel never pays.
                    hints = (mybir.EngineType.PE, mybir.EngineType.DVE,
                             mybir.EngineType.SP, mybir.EngineType.Activation,
                             mybir.EngineType.Pool)
                    with tc.For_i(0, loop_iters, 1, hint_engines=hints):
                        body()
                else:
                    body()
                import concourse.mybir as _mybir  # noqa: F401
                dn = wpool.tile([1, 1], F32, name="dn")
                nc.vector.memset(dn[:], 1.0)
                nc.sync.dma_start(out=done_d[:], in_=dn[:])
            else:
                body()

    nc.compile()
    return nc


def _get_nc(**kw):
    key = tuple(sorted(kw.items()))
    if key not in _RUNNER_CACHE:
        _RUNNER_CACHE[key] = _build_nc(**kw)
    return _RUNNER_CACHE[key]


def kernel(x, kernel):
    from concourse.bass_utils import run_bass_kernel_spmd

    x = np.ascontiguousarray(np.asarray(x, dtype=np.float32))
    n_total, C, H, W = x.shape
    assert (n_total, C, H, W) == (16, 256, 256, 256), x.shape
    npc = n_total // N_CORES

    nc = _get_nc(loop_iters=None, n_samples=npc, C=C, **CFG)
    weights = _weight_inputs(np.asarray(kernel, dtype=np.float32))
    in_maps = [
        {"x": x[i * npc : (i + 1) * npc], **weights} for i in range(N_CORES)
    ]
    last_err = None
    for _attempt in range(3):
        try:
            res = run_bass_kernel_spmd(
                nc, in_maps, core_ids=list(range(N_CORES))
            )
            break
        except Exception as e:  # transient NRT/axon device errors; retry
            last_err = e
    else:
        raise last_err
    return np.concatenate([r["y"] for r in res.results], axis=0)



# revision 8
# speedup vs baseline: 1.0048x; 1.0048x over previous
"""nn_Downsample: depthwise 4x4 stride-2 pad-1 blur downsample on 8 NeuronCores.

Input  x [16, 256, 256, 256] fp32 (NCHW), kernel [4, 4] fp32 (rank-1 FIR).
Output   [16, 256, 128, 128] fp32.

Sharding: pure data parallelism - 2 samples per core across 8 cores.

Per-core program: the conv is separable (kernel = fh x fw outer product),
computed per 256x256 plane as two TensorEngine matmul stages:
  stage 1 (contract h): tmpT[w, i] = sum_h x[h, w] * AT[h, i]
  stage 2 (contract w): out[i, j]  = sum_w tmpT[w, i] * B[w, j]
with AT/B banded matrices holding the taps. Stage 1 runs fp32r with the
[W0|W1]/[W1|W0] rhs concat trick keeping the moving dim at 256 (full PE
rate); the two accumulation groups for adjacent w-blocks write overlapping
PSUM ranges so their valid halves land adjacent and one [128,256] DVE copy
replaces two [128,128] copies.

Stage 2 (s2_mode="qsplit") runs bf16 with AT's columns permuted so tmpT's
free dim is (channel, w-half, i%4, i//4); per 4-channel group, 8 matmuls
(4 q-phases x 2 w-halves, 3D lhs AP spanning the channels) produce an
output tile whose partition is (c, i//4) and free is (i%4, j) - i.e. each
partition holds 4 consecutive output rows of one channel, so the store
descriptors are 2 KB (vs 512 B for the natural i-partitioned layout), and
stage-2 PE time halves (bf16 runs full rate at moving dim 128, so none of
the 2x-redundant concat work is needed).

DMA routing: all input loads ride the sync HWDGE ring, all output stores the
scalar HWDGE ring, PSUM->SBUF copies run on DVE only. Mixing loads+stores on
one ring (or putting copies on ACT, which also issues stores) head-of-line
blocks the input stream; measured ~8% slower.
"""

import sys

sys.path.insert(0, "/opt/trn_rl_repo")

import ml_dtypes
import numpy as np

N_CORES = 8

# Final configuration (picked by A/B timing on hardware; see work/exp.py)
CFG = dict(
    c_group=8,
    copy_policy="vec",
    in_dma="sync",
    out_dma="scalar",
    s2_mode="qsplit",
    split_in=2,
    in_qw_merge=True,
    xbufs=6,
    obufs=6,
    tbufs=8,
    ps1bufs=4,
    ps2bufs=4,
)

_RUNNER_CACHE = {}


def _factor_kernel(k):
    k = np.asarray(k, dtype=np.float64)
    canon = np.outer([1.0, 3.0, 3.0, 1.0], [1.0, 3.0, 3.0, 1.0]) / 64.0
    if np.allclose(k, canon, rtol=1e-5, atol=1e-8):
        f = np.array([1.0, 3.0, 3.0, 1.0]) / 8.0
        return f, f
    u, s, vt = np.linalg.svd(k)
    fh = u[:, 0] * np.sqrt(s[0])
    fw = vt[0] * np.sqrt(s[0])
    if fh.sum() < 0:
        fh, fw = -fh, -fw
    return fh, fw


def _band_matrices(fh, fw, H=256, W=256):
    HO, WO = H // 2, W // 2
    AT = np.zeros((H, HO), dtype=np.float32)
    for i in range(HO):
        for a in range(4):
            h = 2 * i - 1 + a
            if 0 <= h < H:
                AT[h, i] = fh[a]
    B = np.zeros((W, WO), dtype=np.float32)
    for j in range(WO):
        for b in range(4):
            w = 2 * j - 1 + b
            if 0 <= w < W:
                B[w, j] = fw[b]
    return AT, B


def _weight_inputs(kernel):
    fh, fw = _factor_kernel(kernel)
    AT, B = _band_matrices(fh, fw)
    B0, B1 = B[:128], B[128:]
    ATe, ATo = AT[0::2], AT[1::2]
    # qsplit column order: position k = q*32 + a holds output row i = 4a + q
    perm = np.array([4 * (k % 32) + k // 32 for k in range(128)])
    ATeq, AToq = ATe[:, perm], ATo[:, perm]
    return {
        "AT01": np.ascontiguousarray(np.concatenate([ATe, ATo], axis=1)),
        "AT10": np.ascontiguousarray(np.concatenate([ATo, ATe], axis=1)),
        "B01": np.ascontiguousarray(np.concatenate([B0, B1], axis=1)),
        "B10": np.ascontiguousarray(np.concatenate([B1, B0], axis=1)),
        "AT01q": np.ascontiguousarray(np.concatenate([ATeq, AToq], axis=1)),
        "AT10q": np.ascontiguousarray(np.concatenate([AToq, ATeq], axis=1)),
        "B0h": np.ascontiguousarray(B0.astype(ml_dtypes.bfloat16)),
        "B1h": np.ascontiguousarray(B1.astype(ml_dtypes.bfloat16)),
    }


def _wnames(s2_mode):
    if s2_mode == "qsplit":
        return ["AT01q", "AT10q", "B0h", "B1h"]
    return ["AT01", "AT10", "B01", "B10"]


def _build_nc(
    *,
    loop_iters=None,
    c_group=8,
    copy_policy="vec",
    in_dma="sync",
    out_dma="scalar",
    s2_mode="qsplit",
    split_in=1,
    in_qw_merge=True,
    xbufs=4,
    obufs=4,
    tbufs=8,
    ps1bufs=4,
    ps2bufs=4,
    n_samples=2,
    C=256,
):
    import concourse.mybir as mybir
    import concourse.tile as tile
    from concourse import bacc

    F32 = mybir.dt.float32
    F32R = mybir.dt.float32r
    BF16 = mybir.dt.bfloat16

    H = W = 256
    HO = WO = 128
    s1_dt = F32R
    s2_dt = BF16 if s2_mode == "qsplit" else F32R
    timing = loop_iters is not None

    nc = bacc.Bacc("TRN2", target_bir_lowering=False)
    if timing:
        x_d = nc.dram_tensor("x", [n_samples, C, H, W], F32, kind="Internal")
        y_d = nc.dram_tensor("y", [n_samples, C, HO, WO], F32, kind="Internal")
        done_d = nc.dram_tensor("done", [1, 1], F32, kind="ExternalOutput")
    else:
        x_d = nc.dram_tensor("x", [n_samples, C, H, W], F32, kind="ExternalInput")
        y_d = nc.dram_tensor("y", [n_samples, C, HO, WO], F32,
                             kind="ExternalOutput")

    wnames = _wnames(s2_mode)
    w_d = {}
    for n in wnames:
        if n.startswith("B") and n.endswith("h"):
            w_d[n] = nc.dram_tensor(n, [128, 128], BF16, kind="ExternalInput")
        else:
            w_d[n] = nc.dram_tensor(n, [128, 256], F32, kind="ExternalInput")

    with tile.TileContext(nc) as tc:
        with (
            tc.tile_pool(name="wpool", bufs=1) as wpool,
            tc.tile_pool(name="xpool", bufs=xbufs) as xpool,
            tc.tile_pool(name="tpool", bufs=tbufs) as tpool,
            tc.tile_pool(name="opool", bufs=obufs) as opool,
            tc.tile_pool(name="ps1", bufs=ps1bufs, space="PSUM") as ps1pool,
            tc.tile_pool(name="ps2", bufs=ps2bufs, space="PSUM") as ps2pool,
        ):
            wt = {}
            for n in wnames:
                if n.startswith("B") and n.endswith("h"):
                    t = wpool.tile([128, 128], BF16, tag=f"w_{n}", name=f"w_{n}")
                    nc.gpsimd.dma_start(out=t[:], in_=w_d[n][:])
                else:
                    wdt = s1_dt if n.startswith("AT") else s2_dt
                    t = wpool.tile([128, 256], wdt, tag=f"w_{n}", name=f"w_{n}")
                    nc.gpsimd.dma_start(out=t[:], in_=w_d[n][:].bitcast(wdt))
                wt[n] = t

            eng_i = 0

            def copy_tile(out_ap, in_ap, kind="tmp"):
                nonlocal eng_i
                if copy_policy == "vec":
                    nc.vector.tensor_copy(out=out_ap, in_=in_ap)
                elif copy_policy == "vec_out_scalar":
                    if kind == "out":
                        nc.scalar.copy(out_ap, in_ap)
                    else:
                        nc.vector.tensor_copy(out=out_ap, in_=in_ap)
                elif copy_policy == "alt":
                    if eng_i % 2 == 0:
                        nc.vector.tensor_copy(out=out_ap, in_=in_ap)
                    else:
                        nc.scalar.copy(out_ap, in_ap)
                    eng_i += 1
                else:
                    raise ValueError(copy_policy)

            def in_engine(cg):
                if in_dma == "alt":
                    return nc.sync if cg % 2 == 0 else nc.scalar
                return getattr(nc, in_dma)

            def out_engine(cg):
                if out_dma == "alt":
                    return nc.sync if cg % 2 == 0 else nc.scalar
                return getattr(nc, out_dma)

            def body():
                for n in range(n_samples):
                    for cg in range(C // c_group):
                        c0 = cg * c_group
                        xt = xpool.tile([128, c_group * 2 * W], s1_dt, tag="X",
                                        name=f"x_{n}_{cg}")
                        cpp = c_group // split_in
                        for sp in range(split_in):
                            cl, ch = sp * cpp, (sp + 1) * cpp
                            src = x_d[n, c0 + cl : c0 + ch, :, :]
                            if in_qw_merge:
                                # explicit (q w) merge: the AP normalizer
                                # does not coalesce the two adjacent rows
                                # itself; 2KB descriptors measure ~5% faster
                                src = src.rearrange(
                                    "c (p q) w -> p c (q w)", q=2)
                            else:
                                src = src.rearrange(
                                    "c (p q) w -> p c q w", q=2)
                            in_engine(cg).dma_start(
                                out=xt[:, cl * 2 * W : ch * 2 * W],
                                in_=src.bitcast(s1_dt))

                        if s2_mode == "qsplit":
                            for g in range(c_group // 4):
                                tmpT = tpool.tile([128, 1024], BF16,
                                                  tag="tmpT",
                                                  name=f"t_{n}_{cg}_{g}")
                                for cl4 in range(4):
                                    c = 4 * g + cl4
                                    pst = ps1pool.tile([128, 384], F32,
                                                       tag="ps1",
                                                       name=f"p1_{n}_{cg}_{c}")
                                    for blk in range(2):
                                        base = c * 2 * W + blk * 128
                                        lhsE = xt[:, base : base + 128]
                                        lhsO = xt[:, base + W : base + W + 128]
                                        dst = pst[:, blk * 128 : blk * 128 + 256]
                                        nc.tensor.matmul(dst, lhsE,
                                                         wt["AT01q"][:],
                                                         start=True, stop=False)
                                        nc.tensor.matmul(dst, lhsO,
                                                         wt["AT10q"][:],
                                                         start=False, stop=True)
                                    # tmpT free layout is (half, q, c, a) so
                                    # each stage-2 lhs slice is contiguous;
                                    # the copy scatters this channel's
                                    # (half, q, a) stripes, casting to bf16
                                    dstT = tmpT[:].rearrange(
                                        "p (h q c a) -> p h q c a",
                                        h=2, q=4, c=4, a=32)[:, :, :, cl4, :]
                                    srcT = pst[:, 0:256].rearrange(
                                        "p (h q a) -> p h q a",
                                        h=2, q=4, a=32)
                                    copy_tile(dstT, srcT)
                                po = ps2pool.tile([128, 512], F32, tag="ps2",
                                                  name=f"p2_{n}_{cg}_{g}")
                                for q in range(4):
                                    dst2 = po[:, q * 128 : q * 128 + 128]
                                    nc.tensor.matmul(dst2,
                                                     tmpT[:, q * 128 : q * 128 + 128],
                                                     wt["B0h"][:],
                                                     start=True, stop=False)
                                    nc.tensor.matmul(dst2,
                                                     tmpT[:, 512 + q * 128 : 512 + q * 128 + 128],
                                                     wt["B1h"][:],
                                                     start=False, stop=True)
                                out_t = opool.tile([128, 512], F32, tag="OUT",
                                                   name=f"o_{n}_{cg}_{g}")
                                copy_tile(out_t[:], po[:], kind="out")
                                dsty = y_d[n, c0 + 4 * g : c0 + 4 * g + 4,
                                           :, :].rearrange(
                                    "c (a q) j -> (c a) (q j)", q=4)
                                out_engine(cg).dma_start(out=dsty,
                                                         in_=out_t[:])
                            continue

                        out_t = opool.tile([128, c_group * WO], F32, tag="OUT",
                                           name=f"o_{n}_{cg}")

                        if s2_mode == "pair":
                            for cp in range(c_group // 2):
                                po = ps2pool.tile([128, 384], F32, tag="ps2",
                                                  name=f"p2_{n}_{cg}_{cp}")
                                for ci in range(2):
                                    c = 2 * cp + ci
                                    pst = ps1pool.tile([128, 384], F32,
                                                       tag="ps1",
                                                       name=f"p1_{n}_{cg}_{c}")
                                    for blk in range(2):
                                        base = c * 2 * W + blk * 128
                                        lhsE = xt[:, base : base + 128]
                                        lhsO = xt[:, base + W : base + W + 128]
                                        dst = pst[:, blk * 128 : blk * 128 + 256]
                                        nc.tensor.matmul(dst, lhsE,
                                                         wt["AT01"][:],
                                                         start=True, stop=False)
                                        nc.tensor.matmul(dst, lhsO,
                                                         wt["AT10"][:],
                                                         start=False, stop=True)
                                    tmpT = tpool.tile([128, 256], s2_dt,
                                                      tag="tmpT",
                                                      name=f"t_{n}_{cg}_{c}")
                                    copy_tile(tmpT[:], pst[:, 0:256])

                                    dst2 = po[:, ci * 128 : ci * 128 + 256]
                                    nc.tensor.matmul(dst2, tmpT[:, 0:128],
                                                     wt["B01"][:],
                                                     start=True, stop=False)
                                    nc.tensor.matmul(dst2, tmpT[:, 128:256],
                                                     wt["B10"][:],
                                                     start=False, stop=True)
                                c0p = 2 * cp * WO
                                copy_tile(out_t[:, c0p : c0p + 2 * WO],
                                          po[:, 0:256], kind="out")
                        else:
                            for c in range(c_group):
                                pst = ps1pool.tile([128, 512], F32, tag="ps1",
                                                   name=f"p1_{n}_{cg}_{c}")
                                for blk in range(2):
                                    base = c * 2 * W + blk * 128
                                    lhsE = xt[:, base : base + 128]
                                    lhsO = xt[:, base + W : base + W + 128]
                                    dst = pst[:, blk * 256 : blk * 256 + 256]
                                    nc.tensor.matmul(dst, lhsE, wt["AT01"][:],
                                                     start=True, stop=False)
                                    nc.tensor.matmul(dst, lhsO, wt["AT10"][:],
                                                     start=False, stop=True)
                                tmpT = tpool.tile([128, 256], s2_dt, tag="tmpT",
                                                  name=f"t_{n}_{cg}_{c}")
                                copy_tile(tmpT[:, 0:128], pst[:, 0:128])
                                copy_tile(tmpT[:, 128:256], pst[:, 256:384])

                                po = ps2pool.tile([128, 256], F32, tag="ps2",
                                                  name=f"p2_{n}_{cg}_{c}")
                                nc.tensor.matmul(po[:], tmpT[:, 0:128],
                                                 wt["B01"][:],
                                                 start=True, stop=False)
                                nc.tensor.matmul(po[:], tmpT[:, 128:256],
                                                 wt["B10"][:],
                                                 start=False, stop=True)

                                copy_tile(out_t[:, c * WO : c * WO + WO],
                                          po[:, 0:128], kind="out")

                        dsty = y_d[n, c0 : c0 + c_group, :, :].rearrange(
                            "c i j -> i c j")
                        out_engine(cg).dma_start(out=dsty, in_=out_t[:])

            if timing:
                if loop_iters > 1:
                    # The body spans many IRAM blocks per engine; without
                    # branch hints every back-edge pays a ~3-4us I$-miss
                    # stall that the single-shot (graded) kernel never pays.
                    hints = (mybir.EngineType.PE, mybir.EngineType.DVE,
                             mybir.EngineType.SP, mybir.EngineType.Activation,
                             mybir.EngineType.Pool)
                    with tc.For_i(0, loop_iters, 1, hint_engines=hints):
                        body()
                else:
                    body()
                import concourse.mybir as _mybir  # noqa: F401
                dn = wpool.tile([1, 1], F32, name="dn")
                nc.vector.memset(dn[:], 1.0)
                nc.sync.dma_start(out=done_d[:], in_=dn[:])
            else:
                body()

    nc.compile()
    return nc


def _get_nc(**kw):
    key = tuple(sorted(kw.items()))
    if key not in _RUNNER_CACHE:
        _RUNNER_CACHE[key] = _build_nc(**kw)
    return _RUNNER_CACHE[key]


def kernel(x, kernel):
    from concourse.bass_utils import run_bass_kernel_spmd

    x = np.ascontiguousarray(np.asarray(x, dtype=np.float32))
    n_total, C, H, W = x.shape
    assert (n_total, C, H, W) == (16, 256, 256, 256), x.shape
    npc = n_total // N_CORES

    nc = _get_nc(loop_iters=None, n_samples=npc, C=C, **CFG)
    weights = _weight_inputs(np.asarray(kernel, dtype=np.float32))
    weights = {k: weights[k] for k in _wnames(CFG["s2_mode"])}
    in_maps = [
        {"x": x[i * npc : (i + 1) * npc], **weights} for i in range(N_CORES)
    ]
    last_err = None
    for _attempt in range(3):
        try:
            res = run_bass_kernel_spmd(
                nc, in_maps, core_ids=list(range(N_CORES))
            )
            break
        except Exception as e:  # transient NRT/axon device errors; retry
            last_err = e
    else:
        raise last_err
    return np.concatenate([r["y"] for r in res.results], axis=0)

